# revision 1
# baseline (speedup 1.0000x reference)
"""Bass/Trainium2 kernel for a 2-layer LSTM (B=512, T=2048, I=3, H=64).

Returns the final hidden state of layer 2, shape (512, 64) fp32.

Strategy (data-parallel over batch, 8 cores x 64 batch each):
  - All recurrent state lives in SBUF for the whole T=2048 recurrence.
  - State convention: ht = 2*h stored transposed (H on partitions, batch on
    free dim) in one (128, BL) fp16 tile: rows 0-63 = ht1 (layer1),
    rows 64-127 = ht2 (layer2).  Weights that multiply ht carry a 0.5.
  - sigmoid(z) = (tanh(z/2)+1)/2: the 0.5 is baked into the i/f/o gate
    weights, so ONE tanh ACTIVATE covers all four gates of a layer.
  - Cell state kept as c2x = 2*c in fp32; tanh(c) = tanh(0.5*c2x) via the
    ACT scale field.
  - x and the biases enter through a K=4 matmul (rows: x0,x1,x2,ones) from
    a host-pretransposed (4, T*BL) fp16 tensor, DMA'd in chunks.
  - The two layers run staggered by one timestep as two interleaved
    dependency chains.

Gate algebra per layer per step (i,f,g,o; ti=tanh(zi/2) etc, tg=tanh(zg)):
  u   = (ti + 1) * tg          # = 2*i*g            scalar_tensor_tensor
  w   = (0.5*c2x) * tf         # = tf*c             scalar_tensor_tensor
  s   = u + w                                        tensor_tensor
  c2x = 0.5*c2x + s            # = 2(f*c + i*g)     scalar_tensor_tensor
  tc  = tanh(0.5*c2x)                                ACT
  ht  = (to + 1) * tc          # = 2*o*tanh(c)      scalar_tensor_tensor
"""

import numpy as np

B, T, I, H = 512, 2048, 3, 64
NCORES = 8
BL = B // NCORES  # 64 batch per core
CH = 64  # timesteps per x-chunk DMA

_CACHE = {}


def _prep_weights(W_ih0, W_hh0, b_ih0, b_hh0, W_ih1, W_hh1, b_ih1, b_hh1):
    """Pack host-side lhsT weight arrays (fp16).

    Column order within each 256-col block: [i(64) | f(64) | g(64) | o(64)],
    i.e. if-block = cols 0..127, go-block = cols 128..255.
    """
    sg = np.concatenate(
        [np.full(H, 0.5), np.full(H, 0.5), np.full(H, 1.0), np.full(H, 0.5)]
    ).astype(np.float32)  # tanh-arg scale per gate row (i,f,g,o)

    b0 = (b_ih0 + b_hh0) * sg
    b1 = (b_ih1 + b_hh1) * sg
    Wx0 = W_ih0 * sg[:, None]  # acts on true x
    Wh0 = W_hh0 * sg[:, None] * 0.5  # acts on ht1 = 2*h1
    Wi1 = W_ih1 * sg[:, None] * 0.5  # acts on ht1
    Wh1 = W_hh1 * sg[:, None] * 0.5  # acts on ht2

    # Gate column order: layer 1 uses [f,i,o,g] so its elementwise algebra is
    # partition-aligned in rows 0-63; layer 2 uses [i,f,g,o] (aligned in rows
    # 64-127).  See cell_update.
    p1 = np.r_[H : 2 * H, 0:H, 3 * H : 4 * H, 2 * H : 3 * H]

    # w13: (68, 512).  cols 0-255: layer-1 lhsT (state rows 0-63, x rows
    # 64-66, bias row 67).  cols 256-511: layer-2 x-block lhsT (rows 64-66
    # zero, row 67 = layer-2 bias) -- rides the same K=4 rhs.
    w13 = np.zeros((68, 512), np.float32)
    w13[0:64, 0:256] = Wh0.T[:, p1]
    w13[64:67, 0:256] = Wx0.T[:, p1]
    w13[67, 0:256] = b0[p1]
    w13[67, 256:512] = b1
    # w2: (128, 256) layer-2 state lhsT: rows 0-63 act on ht1, 64-127 on ht2.
    w2 = np.concatenate([Wi1.T, Wh1.T], axis=0)
    return w13.astype(np.float16), np.ascontiguousarray(w2).astype(np.float16)


def build_program(t_steps=T, bl=BL):
    """Build the Bass program (one core's SPMD program)."""
    import concourse.bass as bass
    import concourse.tile as tile
    from concourse import bacc, mybir

    f32 = mybir.dt.float32
    f16 = mybir.dt.float16
    Tanh = mybir.ActivationFunctionType.Tanh
    ADD = mybir.AluOpType.add
    MULT = mybir.AluOpType.mult

    nc = bacc.Bacc("TRN2", target_bir_lowering=False, debug=False)

    xt_d = nc.dram_tensor("xt", [4, t_steps * bl], f16, kind="ExternalInput")
    w13_d = nc.dram_tensor("w13", [68, 512], f16, kind="ExternalInput")
    w2_d = nc.dram_tensor("w2", [128, 256], f16, kind="ExternalInput")
    out_d = nc.dram_tensor("out", [64, bl], f32, kind="ExternalOutput")

    n_chunks = (t_steps + CH - 1) // CH

    with tile.TileContext(nc) as tc:
        with (
            tc.tile_pool(name="const", bufs=1) as constp,
            tc.tile_pool(name="xchunk", bufs=2) as xpool,
            tc.tile_pool(name="gates", bufs=4) as gpool,
            tc.tile_pool(name="scratch", bufs=4) as spool,
            tc.tile_pool(name="ps1", bufs=3, space="PSUM") as ps1pool,
            tc.tile_pool(name="ps2", bufs=3, space="PSUM") as ps2pool,
        ):
            # --- constants / persistent state ---
            w13 = constp.tile([68, 512], f16, tag="w13")
            nc.sync.dma_start(w13[:, :], w13_d.ap()[:, :])
            w2 = constp.tile([128, 256], f16, tag="w2")
            nc.sync.dma_start(w2[:, :], w2_d.ap()[:, :])

            st = constp.tile([128, bl], f16, tag="state")  # [ht1; ht2]
            nc.vector.memset(st[:, :], 0.0)
            c1t = constp.tile([128, bl], f32, tag="c1")  # c2x layer1 (rows 0-63)
            nc.vector.memset(c1t[:, :], 0.0)
            c2t = constp.tile([128, bl], f32, tag="c2")  # c2x layer2 (rows 64-127)
            nc.vector.memset(c2t[:, :], 0.0)
            c1 = c1t[0:64, :]
            c2 = c2t[64:128, :]

            x_tiles = [None] * n_chunks

            def get_xchunk(ci):
                if x_tiles[ci] is None:
                    xt = xpool.tile([128, CH * bl], f16, tag="x")
                    lo = ci * CH * bl
                    hi = min((ci + 1) * CH, t_steps) * bl
                    nc.sync.dma_start(xt[64:68, 0 : hi - lo], xt_d.ap()[:, lo:hi])
                    x_tiles[ci] = xt
                return x_tiles[ci]

            def xslice(t):
                ci, off = divmod(t, CH)
                return get_xchunk(ci)[64:68, off * bl : (off + 1) * bl]

            # Per-layer step state handles
            ps2_of = {}  # step -> psum tile of layer-2 gates

            def l1_mms(t):
                """Layer-1 gate matmuls for step t -> psum (128, 2*bl)."""
                ps = ps1pool.tile([128, 512], f32, tag="ps1", name="ps1")[:, 0 : 2 * bl]
                xr = xslice(t)
                nc.tensor.matmul(ps[:, 0:bl], w13[64:68, 0:128], xr,
                                 start=True, stop=False)
                nc.tensor.matmul(ps[:, bl : 2 * bl], w13[64:68, 128:256], xr,
                                 start=False, stop=False)
                nc.tensor.matmul(ps[:, 0:bl], w13[0:64, 0:128], st[0:64, :],
                                 start=False, stop=False)
                nc.tensor.matmul(ps[:, bl : 2 * bl], w13[0:64, 128:256],
                                 st[0:64, :], start=False, stop=True)
                return ps

            def l2_mms(t):
                """Layer-2 gate matmuls for step t (needs ht1(t), ht2(t-1))."""
                ps = ps2pool.tile([128, 512], f32, tag="ps2", name="ps2")[:, 0 : 2 * bl]
                xr = xslice(t)  # only the ones-row matters (rows 64-66 hit zeros)
                nc.tensor.matmul(ps[:, 0:bl], w13[64:68, 256:384], xr,
                                 start=True, stop=False)
                nc.tensor.matmul(ps[:, bl : 2 * bl], w13[64:68, 384:512], xr,
                                 start=False, stop=False)
                nc.tensor.matmul(ps[:, 0:bl], w2[:, 0:128], st[:, :],
                                 start=False, stop=False)
                nc.tensor.matmul(ps[:, bl : 2 * bl], w2[:, 128:256], st[:, :],
                                 start=False, stop=True)
                ps2_of[t] = ps

            def slices_of(t1, layer):
                """Layer 1 gate col order [f,i,o,g]: algebra rows 0-63.
                Layer 2 gate col order [i,f,g,o]: algebra rows 64-127."""
                if layer == 1:
                    lo = slice(0, 64)
                    tf, ti = t1[0:64, 0:bl], t1[64:128, 0:bl]
                    to, tg = t1[0:64, bl : 2 * bl], t1[64:128, bl : 2 * bl]
                else:
                    lo = slice(64, 128)
                    ti, tf = t1[0:64, 0:bl], t1[64:128, 0:bl]
                    tg, to = t1[0:64, bl : 2 * bl], t1[64:128, bl : 2 * bl]
                return lo, ti, tf, tg, to

            def cell_a(ps, layer):
                """ACT: tanh over all four gate blocks."""
                t1 = gpool.tile([128, 2 * bl], f16, tag=f"t1l{layer}",
                                name=f"t1l{layer}")
                nc.scalar.activation(t1[:, :], ps[:, :], Tanh)
                return t1

            def cell_b(t1, cc, layer):
                """DVE cell update in 3 ops:
                u = (ti+1)*tg = 2ig;  w = (tf+1)*c2x = 4fc;
                c2x = 0.5*w + u = 2(fc + ig)."""
                lo, ti, tf, tg, to = slices_of(t1, layer)
                u = spool.tile([128, bl], f16, tag=f"u{layer}", name=f"u{layer}")[lo, :]
                nc.vector.scalar_tensor_tensor(u, ti, 1.0, tg, ADD, MULT)
                w = spool.tile([128, bl], f32, tag=f"w{layer}", name=f"w{layer}")[lo, :]
                nc.vector.scalar_tensor_tensor(w, tf, 1.0, cc, ADD, MULT)
                nc.vector.scalar_tensor_tensor(cc, w, 0.5, u, MULT, ADD)

            def cell_c(t1, cc, layer):
                """ACT tanh(c) + DVE ht = (to+1)*tc -> st."""
                lo, ti, tf, tg, to = slices_of(t1, layer)
                tcl = spool.tile([128, bl], f16, tag=f"tc{layer}",
                                 name=f"tc{layer}")[lo, :]
                nc.scalar.activation(tcl, cc, Tanh, scale=0.5)
                nc.vector.scalar_tensor_tensor(st[lo, :], to, 1.0, tcl, ADD, MULT)

            # Emission order = per-engine queue order.  Interleave the two
            # layer chains (L2 runs one step behind L1) so neither chain
            # head-of-line-blocks the other on the ACT/DVE FIFOs.
            for t in range(t_steps):
                ps1 = l1_mms(t)
                if t >= 1:
                    l2_mms(t - 1)
                t1b = cell_a(ps2_of.pop(t - 1), 2) if t >= 1 else None
                t1a = cell_a(ps1, 1)
                if t1b is not None:
                    cell_b(t1b, c2, 2)
                cell_b(t1a, c1, 1)
                if t1b is not None:
                    cell_c(t1b, c2, 2)  # writes ht2(t-1)
                cell_c(t1a, c1, 1)  # writes ht1(t)
                # free old x chunk handle (keeps python refs bounded)
                ci = t // CH
                if ci >= 2:
                    x_tiles[ci - 2] = None

            l2_mms(t_steps - 1)
            t1b = cell_a(ps2_of.pop(t_steps - 1), 2)
            cell_b(t1b, c2, 2)
            cell_c(t1b, c2, 2)

            # out = 0.5 * ht2 = h2_final (transposed: H x batch), fp32
            ob = constp.tile([128, bl], f32, tag="out")
            nc.vector.tensor_scalar_mul(ob[64:128, :], st[64:128, :], 0.5)
            nc.sync.dma_start(out_d.ap()[:, :], ob[64:128, :])

    nc.compile()
    return nc


def _get_program(t_steps=T):
    key = ("prog", t_steps)
    if key not in _CACHE:
        _CACHE[key] = build_program(t_steps)
    return _CACHE[key]


def kernel(x, W_ih0, W_hh0, b_ih0, b_hh0, W_ih1, W_hh1, b_ih1, b_hh1):
    from concourse import bass_utils

    x = np.asarray(x, np.float32)
    w13, w2 = _prep_weights(
        np.asarray(W_ih0, np.float32), np.asarray(W_hh0, np.float32),
        np.asarray(b_ih0, np.float32), np.asarray(b_hh0, np.float32),
        np.asarray(W_ih1, np.float32), np.asarray(W_hh1, np.float32),
        np.asarray(b_ih1, np.float32), np.asarray(b_hh1, np.float32),
    )

    nc = _get_program(T)

    in_maps = []
    for c in range(NCORES):
        xc = x[c * BL : (c + 1) * BL]  # (BL, T, 3)
        xt = np.ones((4, T * BL), np.float16)
        xt[0:3] = xc.transpose(2, 1, 0).reshape(3, T * BL).astype(np.float16)
        in_maps.append({"xt": xt, "w13": w13, "w2": w2})

    res = bass_utils.run_bass_kernel_spmd(nc, in_maps, core_ids=list(range(NCORES)))
    outs = [res.results[c]["out"].T for c in range(NCORES)]  # (BL, 64) each
    return np.concatenate(outs, axis=0).astype(np.float32)


if __name__ == "__main__":
    rng = np.random.default_rng(0)
    s = 1.0 / np.sqrt(H)
    inputs = {
        "x": rng.standard_normal((B, T, I), np.float32),
        "W_ih0": rng.uniform(-s, s, (4 * H, I)).astype(np.float32),
        "W_hh0": rng.uniform(-s, s, (4 * H, H)).astype(np.float32),
        "b_ih0": rng.uniform(-s, s, 4 * H).astype(np.float32),
        "b_hh0": rng.uniform(-s, s, 4 * H).astype(np.float32),
        "W_ih1": rng.uniform(-s, s, (4 * H, H)).astype(np.float32),
        "W_hh1": rng.uniform(-s, s, (4 * H, H)).astype(np.float32),
        "b_ih1": rng.uniform(-s, s, 4 * H).astype(np.float32),
        "b_hh1": rng.uniform(-s, s, 4 * H).astype(np.float32),
    }
    out = kernel(**inputs)
    print(out.shape, out.dtype, np.abs(out).max())



# revision 3
# speedup vs baseline: 1.4037x; 1.4037x over previous
"""Bass/Trainium2 kernel for a 2-layer LSTM (B=512, T=2048, I=3, H=64).

Raw-bass (no TileContext) fused-layer design with hand-rolled semaphores:
every engine instruction carries at most ONE semaphore wait (its critical
RAW dependency); all WAR hazards are covered transitively through the
chain structure, so no standalone EventSemaphore instructions are needed
in steady state and the real data wait rides on the instruction itself
(parking in the engine wait queue instead of blocking the sequencer).

Math and layout identical to the tile version (kernel2):
  - layers fused on partitions: L1 rows 0-63, L2 rows 64-127; L2 lags one
    step.  Gate PSUM colblocks f,i,g,o; K-stacked state matmuls on
    st=[2h1;2h2] (fp16); c2x=2c (fp32).
  - per tick/group: u=(ti+1)tg; w=(tf+1)c2x; c2x=0.5w+u; tc=tanh(0.5c2x);
    st=(to+1)tc.

Sync plan per tick t, group g (sems: sP=PE matmuls, sA=ACT, sD=DVE,
sM=DMA):
  x-MMs:      no wait (chunk-boundary first MM waits sM)
  state-MM0:  wait sD >= ht(t-1,g)      (MM1-3: none, in-order)
  gates ACT:  wait sP >= stateMM3(t,g)
  u:          wait sA >= gates(t,g)
  w:          none (in-order after u)
  c:          wait sD >= w(t,g)         (same-engine pipelined-write)
  tc ACT:     wait sD >= c(t,g)
  ht:         wait sA >= tc(t,g)
  chunk DMA:  wait sP >= last x-MM of the buffer's previous tenant
"""

import numpy as np

B, T, I, H = 512, 2048, 3, 64
NCORES = 8
BL = B // NCORES  # 64 batch per core
BGS = [16, 24, 24]  # batch-group sizes (independent chains per core)
G = len(BGS)
BOFF = [sum(BGS[:g]) for g in range(G)]
CH = 64  # timesteps per x-chunk DMA

_CACHE = {}

_GATES = ["f", "i", "g", "o"]  # colblock order
_ROWS = {"i": slice(0, H), "f": slice(H, 2 * H), "g": slice(2 * H, 3 * H),
         "o": slice(3 * H, 4 * H)}
_SG = {"i": 0.5, "f": 0.5, "g": 1.0, "o": 0.5}


def _prep_weights(W_ih0, W_hh0, b_ih0, b_hh0, W_ih1, W_hh1, b_ih1, b_hh1):
    """Pack host-side lhsT weights (fp16). See kernel2 docstring."""
    b0 = b_ih0 + b_hh0
    b1 = b_ih1 + b_hh1
    wst = np.zeros((128, 512), np.float32)
    wx = np.zeros((4, 512), np.float32)
    for cb, gate in enumerate(_GATES):
        r = _ROWS[gate]
        sg = _SG[gate]
        c0 = cb * 128
        wst[0:64, c0:c0 + 64] = (W_hh0[r] * sg * 0.5).T
        wx[0:3, c0:c0 + 64] = (W_ih0[r] * sg).T
        wx[3, c0:c0 + 64] = b0[r] * sg
        wst[0:64, c0 + 64:c0 + 128] = (W_ih1[r] * sg * 0.5).T
        wst[64:128, c0 + 64:c0 + 128] = (W_hh1[r] * sg * 0.5).T
        wx[3, c0 + 64:c0 + 128] = b1[r] * sg
    return wst.astype(np.float16), wx.astype(np.float16)


def build_program(t_steps=T, bl=BL):
    from concourse import bacc, mybir

    f32 = mybir.dt.float32
    f16 = mybir.dt.float16
    Tanh = mybir.ActivationFunctionType.Tanh
    ADD = mybir.AluOpType.add
    MULT = mybir.AluOpType.mult

    nc = bacc.Bacc("TRN2", target_bir_lowering=False, debug=False)

    # xt carries a 512-col wx prefix so one DMA lands wx + chunk 0
    xt_d = nc.dram_tensor("xt", [4, 512 + t_steps * bl], f16,
                          kind="ExternalInput")
    wst_d = nc.dram_tensor("wst", [128, 512], f16, kind="ExternalInput")
    out_d = nc.dram_tensor("out", [64, bl], f32, kind="ExternalOutput")

    n_chunks = (t_steps + CH - 1) // CH
    PSR = 2  # psum ring depth per group

    with nc.cleanup_on_exit():
        sP = nc.alloc_semaphore("sP")
        sA = nc.alloc_semaphore("sA")
        sD = nc.alloc_semaphore("sD")
        sM = nc.alloc_semaphore("sM")
        cnt = {"P": 0, "A": 0, "D": 0, "M": 0}

        def inc(inst, which, sem, by=1):
            inst.then_inc(sem, by)
            cnt[which] += by
            return cnt[which]

        # --- sbuf/psum tensors ---
        wst = nc.alloc_sbuf_tensor("wst_s", [128, 512], f16)
        xch = [nc.alloc_sbuf_tensor(f"xch{r}", [4, 512 * (r == 0) + CH * bl],
                                    f16) for r in range(2)]
        wx = xch[0]  # cols 0:512 of xch0, loaded by the first chunk DMA
        # t1e layout: colblocks [f | i | c2x | g | o], each bg wide.  The
        # gates ACT writes [f,i] and [g,o] via one sub-strided AP; c2x (fp16)
        # is owned by the cell update.  [f,i] and [c2x,g] are contiguous
        # operand pairs for the paired cell STT.
        sts = [nc.alloc_sbuf_tensor(f"st{g}", [128, BGS[g]], f16)
               for g in range(G)]
        t1s = [nc.alloc_sbuf_tensor(f"t1{g}", [128, 5 * BGS[g]], f16)
               for g in range(G)]
        uws = [nc.alloc_sbuf_tensor(f"uw{g}", [128, 2 * BGS[g]], f16)
               for g in range(G)]
        tcs = [nc.alloc_sbuf_tensor(f"tc{g}", [128, BGS[g]], f16)
               for g in range(G)]

        import bass_rust as _br

        def gates_out_ap(g):
            """[128, 2, 2bg] view of t1e hitting cols [f,i] then [g,o]."""
            bg = BGS[g]
            a = t1s[g].ap().copy()
            a.ap = _br.VecI64Pair([[5 * bg, 128], [3 * bg, 2], [1, 2 * bg]])
            return a
        ob = nc.alloc_sbuf_tensor("ob", [128, bl], f32)
        pss = [[nc.alloc_psum_tensor(f"ps{g}_{r}", [128, 4 * BGS[g]], f32)
                for r in range(PSR)] for g in range(G)]

        # --- preload first chunk + weights (chunk0 first: HWDGE issues
        # serially, so the tick-0 critical path clears ~780ns sooner; the
        # first x-MM's single sM wait at the post-wx count transitively
        # covers chunk0/wst/wx since DMA sem counts are emission-ordered) ---
        chunk_dma_count = [None] * n_chunks  # sM count when chunk ci loaded
        chunk_last_reader = [0] * 2  # sP count of last x-MM using buffer r

        def fetch_chunk(ci, wait_pe=None):
            # chunk ci source starts after the 512-col wx prefix; chunk 0's
            # DMA spans the prefix too, landing wx and chunk 0 together.
            # Even chunks reload xch0 cols 512: only, keeping wx intact.
            lo = 512 + ci * CH * bl
            hi = 512 + min((ci + 1) * CH, t_steps) * bl
            pre = 512 if ci == 0 else 0
            base = 0 if ci == 0 else 512 * (ci % 2 == 0)
            d = nc.sync.dma_start(
                xch[ci % 2].ap()[0:4, base:base + pre + hi - lo],
                xt_d.ap()[:, lo - pre:hi])
            if wait_pe:
                d._wait_ge(sP, wait_pe)
            chunk_dma_count[ci] = inc(d, "M", sM, 16)

        inc(nc.sync.dma_start(wst.ap()[:, :], wst_d.ap()[:, :]), "M", sM, 16)
        fetch_chunk(0)
        preload_cnt = cnt["M"]  # wst + (wx+chunk0) landed at this count
        if n_chunks > 1:
            fetch_chunk(1)

        # initial state zeroing (DVE, counted)
        for g in range(G):
            bg = BGS[g]
            inc(nc.vector.memset(sts[g].ap()[:, :], 0.0), "D", sD)
            inc(nc.vector.memset(t1s[g].ap()[:, 2 * bg:3 * bg], 0.0), "D", sD)

        ht_cnt = [cnt["D"]] * G  # sD count after ht(t-1, g) (init: memsets)
        first_mm = True

        def xslice(t, g):
            ci, off = divmod(t, CH)
            base = 512 * (ci % 2 == 0) + off * bl + BOFF[g]
            return xch[ci % 2].ap()[0:4, base:base + BGS[g]]

        for t in range(t_steps + 1):
            ci = t // CH
            if t % CH == 1 and ci + 1 < n_chunks:
                # prefetch next chunk into the buffer last used by ci-1
                fetch_chunk(ci + 1,
                            wait_pe=chunk_last_reader[(ci + 1) % 2] or None)

            mm_cnt = [0] * G
            g_cnt = [0] * G
            c_cnt = [0] * G
            t_cnt = [0] * G
            for g in range(G):
                bg = BGS[g]
                ps = pss[g][t % PSR].ap()
                xr = xslice(min(t, t_steps - 1), g)
                st = sts[g].ap()
                # x matmuls (start accumulation)
                for cb in range(4):
                    mm = nc.tensor.matmul(ps[:, cb * bg:(cb + 1) * bg],
                                          wx.ap()[0:4,
                                                  cb * 128:(cb + 1) * 128],
                                          xr, start=cb == 0, stop=False)
                    if first_mm:
                        mm._wait_ge(sM, preload_cnt)
                        first_mm = False
                    inc(mm, "P", sP)
                if t % CH == CH - 1 or t == t_steps:
                    chunk_last_reader[ci % 2] = cnt["P"]
                # state matmuls
                for cb in range(4):
                    mm = nc.tensor.matmul(ps[:, cb * bg:(cb + 1) * bg],
                                          wst.ap()[:, cb * 128:(cb + 1) * 128],
                                          st[:, :], start=False, stop=cb == 3)
                    if cb == 0:
                        mm._wait_ge(sD, ht_cnt[g])
                    inc(mm, "P", sP)
                mm_cnt[g] = cnt["P"]
            for g in range(G):
                bg = BGS[g]
                psv = pss[g][t % PSR].ap()[:, :].rearrange(
                    "p (s n) -> p s n", s=2)
                act = nc.scalar.activation(gates_out_ap(g), psv, Tanh)
                act._wait_ge(sP, mm_cnt[g])
                g_cnt[g] = inc(act, "A", sA)
            def emit_cell(g):
                bg = BGS[g]
                t1 = t1s[g].ap()
                r3 = lambda a: a.rearrange("p (s n) -> p s n", s=2)
                # paired STT: [w|u] = (in+1)*other for pairs (tf,c2x),(ti,tg)
                p1 = nc.vector.scalar_tensor_tensor(
                    r3(uws[g].ap()[:, :]), r3(t1[:, 0:2 * bg]), 1.0,
                    r3(t1[:, 2 * bg:4 * bg]), ADD, MULT)
                p1._wait_ge(sA, g_cnt[g])
                p1_cnt = inc(p1, "D", sD)
                # c2x = 0.5*w + u, written into the t1e c2x block
                cc = nc.vector.scalar_tensor_tensor(
                    t1[:, 2 * bg:3 * bg], uws[g].ap()[:, 0:bg], 0.5,
                    uws[g].ap()[:, bg:2 * bg], MULT, ADD)
                cc._wait_ge(sD, p1_cnt)
                c_cnt[g] = inc(cc, "D", sD)
                ta = nc.scalar.activation(tcs[g].ap()[:, :],
                                          t1[:, 2 * bg:3 * bg],
                                          Tanh, scale=0.5)
                ta._wait_ge(sD, c_cnt[g])
                t_cnt[g] = inc(ta, "A", sA)

            def emit_ht(g):
                bg = BGS[g]
                ht = nc.vector.scalar_tensor_tensor(
                    sts[g].ap()[:, :], t1s[g].ap()[:, 4 * bg:5 * bg], 1.0,
                    tcs[g].ap()[:, :], ADD, MULT)
                ht._wait_ge(sA, t_cnt[g])
                ht_cnt[g] = inc(ht, "D", sD)

            for g in range(G):
                emit_cell(g)
            for g in range(G):
                emit_ht(g)

            if t == 0:
                # wipe layer-2 pollution from the bogus step -1
                for g in range(G):
                    bg = BGS[g]
                    inc(nc.vector.memset(sts[g].ap()[64:128, :], 0.0), "D", sD)
                    inc(nc.vector.memset(
                        t1s[g].ap()[64:128, 2 * bg:3 * bg], 0.0), "D", sD)
                    ht_cnt[g] = cnt["D"]

        # output: h2 = 0.5 * st rows 64:128
        for g in range(G):
            inc(nc.vector.tensor_scalar_mul(
                ob.ap()[64:128, BOFF[g]:BOFF[g] + BGS[g]],
                sts[g].ap()[64:128, :], 0.5), "D", sD)
        od = nc.sync.dma_start(out_d.ap()[:, :], ob.ap()[64:128, :])
        od._wait_ge(sD, cnt["D"])
        od.then_inc(sM, 16)
        cnt["M"] += 16
        nc.sync.wait_ge(sM, cnt["M"])
        nc.all_engine_barrier()

    nc.compile()
    return nc


def _get_program(t_steps=T):
    key = ("prog", t_steps)
    if key not in _CACHE:
        _CACHE[key] = build_program(t_steps)
    return _CACHE[key]


def make_in_map(inputs, core=0):
    x = np.asarray(inputs["x"], np.float32)
    t_steps = x.shape[1]
    wst, wx = _prep_weights(
        *(np.asarray(inputs[k], np.float32) for k in
          ("W_ih0", "W_hh0", "b_ih0", "b_hh0", "W_ih1", "W_hh1", "b_ih1",
           "b_hh1"))
    )
    xc = x[core * BL:(core + 1) * BL]
    xt = np.ones((4, 512 + t_steps * BL), np.float16)
    xt[:, 0:512] = wx
    xt[0:3, 512:] = xc.transpose(2, 1, 0).reshape(3, t_steps * BL).astype(
        np.float16)
    return {"xt": xt, "wst": wst}


def extract_out(out_mem):
    return out_mem.view(np.float32).reshape(64, BL).T.copy()


def kernel(x, W_ih0, W_hh0, b_ih0, b_hh0, W_ih1, W_hh1, b_ih1, b_hh1):
    from concourse import bass_utils

    inputs = dict(x=x, W_ih0=W_ih0, W_hh0=W_hh0, b_ih0=b_ih0, b_hh0=b_hh0,
                  W_ih1=W_ih1, W_hh1=W_hh1, b_ih1=b_ih1, b_hh1=b_hh1)
    nc = _get_program(T)
    in_maps = [make_in_map(inputs, core=c) for c in range(NCORES)]
    res = bass_utils.run_bass_kernel_spmd(nc, in_maps, core_ids=list(range(NCORES)))
    outs = [res.results[c]["out"].T for c in range(NCORES)]
    return np.concatenate(outs, axis=0).astype(np.float32)


if __name__ == "__main__":
    rng = np.random.default_rng(0)
    s = 1.0 / np.sqrt(H)
    inputs = {
        "x": rng.standard_normal((B, T, I), np.float32),
        "W_ih0": rng.uniform(-s, s, (4 * H, I)).astype(np.float32),
        "W_hh0": rng.uniform(-s, s, (4 * H, H)).astype(np.float32),
        "b_ih0": rng.uniform(-s, s, 4 * H).astype(np.float32),
        "b_hh0": rng.uniform(-s, s, 4 * H).astype(np.float32),
        "W_ih1": rng.uniform(-s, s, (4 * H, H)).astype(np.float32),
        "W_hh1": rng.uniform(-s, s, (4 * H, H)).astype(np.float32),
        "b_ih1": rng.uniform(-s, s, 4 * H).astype(np.float32),
        "b_hh1": rng.uniform(-s, s, 4 * H).astype(np.float32),
    }
    out = kernel(**inputs)
    print(out.shape, out.dtype, np.abs(out).max())


# revision 4
# speedup vs baseline: 1.4041x; 1.0003x over previous
"""Bass/Trainium2 kernel for a 2-layer LSTM (B=512, T=2048, I=3, H=64).

Raw-bass (no TileContext) fused-layer design with hand-rolled semaphores:
every engine instruction carries at most ONE semaphore wait (its critical
RAW dependency); all WAR hazards are covered transitively through the
chain structure, so no standalone EventSemaphore instructions are needed
in steady state and the real data wait rides on the instruction itself
(parking in the engine wait queue instead of blocking the sequencer).

Math and layout identical to the tile version (kernel2):
  - layers fused on partitions: L1 rows 0-63, L2 rows 64-127; L2 lags one
    step.  Gate PSUM colblocks f,i,g,o; K-stacked state matmuls on
    st=[2h1;2h2] (fp16); c2x=2c (fp32).
  - per tick/group: u=(ti+1)tg; w=(tf+1)c2x; c2x=0.5w+u; tc=tanh(0.5c2x);
    st=(to+1)tc.

Sync plan per tick t, group g (sems: sP=PE matmuls, sA=ACT, sD=DVE,
sM=DMA):
  x-MMs:      no wait (chunk-boundary first MM waits sM)
  state-MM0:  wait sD >= ht(t-1,g)      (MM1-3: none, in-order)
  gates ACT:  wait sP >= stateMM3(t,g)
  u:          wait sA >= gates(t,g)
  w:          none (in-order after u)
  c:          wait sD >= w(t,g)         (same-engine pipelined-write)
  tc ACT:     wait sD >= c(t,g)
  ht:         wait sA >= tc(t,g)
  chunk DMA:  wait sP >= last x-MM of the buffer's previous tenant
"""

import numpy as np

B, T, I, H = 512, 2048, 3, 64
NCORES = 8
BL = B // NCORES  # 64 batch per core
BGS = [16, 24, 24]  # batch-group sizes (independent chains per core)
G = len(BGS)
BOFF = [sum(BGS[:g]) for g in range(G)]
CH = 64  # timesteps per x-chunk DMA

_CACHE = {}

_GATES = ["f", "i", "g", "o"]  # colblock order
_ROWS = {"i": slice(0, H), "f": slice(H, 2 * H), "g": slice(2 * H, 3 * H),
         "o": slice(3 * H, 4 * H)}
_SG = {"i": 0.5, "f": 0.5, "g": 1.0, "o": 0.5}


def _prep_weights(W_ih0, W_hh0, b_ih0, b_hh0, W_ih1, W_hh1, b_ih1, b_hh1):
    """Pack host-side lhsT weights (fp16). See kernel2 docstring."""
    b0 = b_ih0 + b_hh0
    b1 = b_ih1 + b_hh1
    wst = np.zeros((128, 512), np.float32)
    wx = np.zeros((4, 512), np.float32)
    for cb, gate in enumerate(_GATES):
        r = _ROWS[gate]
        sg = _SG[gate]
        c0 = cb * 128
        wst[0:64, c0:c0 + 64] = (W_hh0[r] * sg * 0.5).T
        wx[0:3, c0:c0 + 64] = (W_ih0[r] * sg).T
        wx[3, c0:c0 + 64] = b0[r] * sg
        wst[0:64, c0 + 64:c0 + 128] = (W_ih1[r] * sg * 0.5).T
        wst[64:128, c0 + 64:c0 + 128] = (W_hh1[r] * sg * 0.5).T
        wx[3, c0 + 64:c0 + 128] = b1[r] * sg
    return wst.astype(np.float16), wx.astype(np.float16)


def build_program(t_steps=T, bl=BL):
    from concourse import bacc, mybir

    f32 = mybir.dt.float32
    f16 = mybir.dt.float16
    Tanh = mybir.ActivationFunctionType.Tanh
    ADD = mybir.AluOpType.add
    MULT = mybir.AluOpType.mult

    nc = bacc.Bacc("TRN2", target_bir_lowering=False, debug=False)

    # xt carries a 512-col wx prefix so one DMA lands wx + chunk 0
    xt_d = nc.dram_tensor("xt", [4, 512 + t_steps * bl], f16,
                          kind="ExternalInput")
    wst_d = nc.dram_tensor("wst", [128, 512], f16, kind="ExternalInput")
    out_d = nc.dram_tensor("out", [64, bl], f16, kind="ExternalOutput")

    n_chunks = (t_steps + CH - 1) // CH
    PSR = 2  # psum ring depth per group

    with nc.cleanup_on_exit():
        sP = nc.alloc_semaphore("sP")
        sA = nc.alloc_semaphore("sA")
        sD = nc.alloc_semaphore("sD")
        sM = nc.alloc_semaphore("sM")
        cnt = {"P": 0, "A": 0, "D": 0, "M": 0}

        def inc(inst, which, sem, by=1):
            inst.then_inc(sem, by)
            cnt[which] += by
            return cnt[which]

        # --- sbuf/psum tensors ---
        wst = nc.alloc_sbuf_tensor("wst_s", [128, 512], f16)
        xch = [nc.alloc_sbuf_tensor(f"xch{r}", [4, 512 * (r == 0) + CH * bl],
                                    f16) for r in range(2)]
        wx = xch[0]  # cols 0:512 of xch0, loaded by the first chunk DMA
        # t1e layout: colblocks [f | i | c2x | g | o], each bg wide.  The
        # gates ACT writes [f,i] and [g,o] via one sub-strided AP; c2x (fp16)
        # is owned by the cell update.  [f,i] and [c2x,g] are contiguous
        # operand pairs for the paired cell STT.
        # one backing tensor for all group states: the final hidden rows
        # DMA out directly (f16; host applies the 0.5 and f32 cast)
        st_all = nc.alloc_sbuf_tensor("st_all", [128, bl], f16)
        sts = [st_all.ap()[:, BOFF[g]:BOFF[g] + BGS[g]] for g in range(G)]
        t1s = [nc.alloc_sbuf_tensor(f"t1{g}", [128, 5 * BGS[g]], f16)
               for g in range(G)]
        uws = [nc.alloc_sbuf_tensor(f"uw{g}", [128, 2 * BGS[g]], f16)
               for g in range(G)]
        tcs = [nc.alloc_sbuf_tensor(f"tc{g}", [128, BGS[g]], f16)
               for g in range(G)]

        import bass_rust as _br

        def gates_out_ap(g):
            """[128, 2, 2bg] view of t1e hitting cols [f,i] then [g,o]."""
            bg = BGS[g]
            a = t1s[g].ap().copy()
            a.ap = _br.VecI64Pair([[5 * bg, 128], [3 * bg, 2], [1, 2 * bg]])
            return a
        pss = [[nc.alloc_psum_tensor(f"ps{g}_{r}", [128, 4 * BGS[g]], f32)
                for r in range(PSR)] for g in range(G)]

        # --- preload first chunk + weights (chunk0 first: HWDGE issues
        # serially, so the tick-0 critical path clears ~780ns sooner; the
        # first x-MM's single sM wait at the post-wx count transitively
        # covers chunk0/wst/wx since DMA sem counts are emission-ordered) ---
        chunk_dma_count = [None] * n_chunks  # sM count when chunk ci loaded
        chunk_last_reader = [0] * 2  # sP count of last x-MM using buffer r

        def fetch_chunk(ci, wait_pe=None):
            # chunk ci source starts after the 512-col wx prefix; chunk 0's
            # DMA spans the prefix too, landing wx and chunk 0 together.
            # Even chunks reload xch0 cols 512: only, keeping wx intact.
            lo = 512 + ci * CH * bl
            hi = 512 + min((ci + 1) * CH, t_steps) * bl
            pre = 512 if ci == 0 else 0
            base = 0 if ci == 0 else 512 * (ci % 2 == 0)
            d = nc.sync.dma_start(
                xch[ci % 2].ap()[0:4, base:base + pre + hi - lo],
                xt_d.ap()[:, lo - pre:hi])
            if wait_pe:
                d._wait_ge(sP, wait_pe)
            chunk_dma_count[ci] = inc(d, "M", sM, 16)

        inc(nc.sync.dma_start(wst.ap()[:, :], wst_d.ap()[:, :]), "M", sM, 16)
        fetch_chunk(0)
        preload_cnt = cnt["M"]  # wst + (wx+chunk0) landed at this count
        if n_chunks > 1:
            fetch_chunk(1)

        # initial state zeroing (DVE, counted)
        for g in range(G):
            bg = BGS[g]
            inc(nc.vector.memset(sts[g][:, :], 0.0), "D", sD)
            inc(nc.vector.memset(t1s[g].ap()[:, 2 * bg:3 * bg], 0.0), "D", sD)

        ht_cnt = [cnt["D"]] * G  # sD count after ht(t-1, g) (init: memsets)
        first_mm = True

        def xslice(t, g):
            ci, off = divmod(t, CH)
            base = 512 * (ci % 2 == 0) + off * bl + BOFF[g]
            return xch[ci % 2].ap()[0:4, base:base + BGS[g]]

        for t in range(t_steps + 1):
            ci = t // CH
            if t % CH == 1 and ci + 1 < n_chunks:
                # prefetch next chunk into the buffer last used by ci-1
                fetch_chunk(ci + 1,
                            wait_pe=chunk_last_reader[(ci + 1) % 2] or None)

            mm_cnt = [0] * G
            g_cnt = [0] * G
            c_cnt = [0] * G
            t_cnt = [0] * G
            for g in range(G):
                bg = BGS[g]
                ps = pss[g][t % PSR].ap()
                xr = xslice(min(t, t_steps - 1), g)
                st = sts[g]
                # x matmuls (start accumulation)
                for cb in range(4):
                    mm = nc.tensor.matmul(ps[:, cb * bg:(cb + 1) * bg],
                                          wx.ap()[0:4,
                                                  cb * 128:(cb + 1) * 128],
                                          xr, start=cb == 0, stop=False)
                    if first_mm:
                        mm._wait_ge(sM, preload_cnt)
                        first_mm = False
                    inc(mm, "P", sP)
                if t % CH == CH - 1 or t == t_steps:
                    chunk_last_reader[ci % 2] = cnt["P"]
                # state matmuls
                for cb in range(4):
                    mm = nc.tensor.matmul(ps[:, cb * bg:(cb + 1) * bg],
                                          wst.ap()[:, cb * 128:(cb + 1) * 128],
                                          st[:, :], start=False, stop=cb == 3)
                    if cb == 0:
                        mm._wait_ge(sD, ht_cnt[g])
                    inc(mm, "P", sP)
                mm_cnt[g] = cnt["P"]
            for g in range(G):
                bg = BGS[g]
                psv = pss[g][t % PSR].ap()[:, :].rearrange(
                    "p (s n) -> p s n", s=2)
                act = nc.scalar.activation(gates_out_ap(g), psv, Tanh)
                act._wait_ge(sP, mm_cnt[g])
                g_cnt[g] = inc(act, "A", sA)
            def emit_cell(g):
                bg = BGS[g]
                t1 = t1s[g].ap()
                r3 = lambda a: a.rearrange("p (s n) -> p s n", s=2)
                # paired STT: [w|u] = (in+1)*other for pairs (tf,c2x),(ti,tg)
                p1 = nc.vector.scalar_tensor_tensor(
                    r3(uws[g].ap()[:, :]), r3(t1[:, 0:2 * bg]), 1.0,
                    r3(t1[:, 2 * bg:4 * bg]), ADD, MULT)
                p1._wait_ge(sA, g_cnt[g])
                p1_cnt = inc(p1, "D", sD)
                # c2x = 0.5*w + u, written into the t1e c2x block
                cc = nc.vector.scalar_tensor_tensor(
                    t1[:, 2 * bg:3 * bg], uws[g].ap()[:, 0:bg], 0.5,
                    uws[g].ap()[:, bg:2 * bg], MULT, ADD)
                cc._wait_ge(sD, p1_cnt)
                c_cnt[g] = inc(cc, "D", sD)
                ta = nc.scalar.activation(tcs[g].ap()[:, :],
                                          t1[:, 2 * bg:3 * bg],
                                          Tanh, scale=0.5)
                ta._wait_ge(sD, c_cnt[g])
                t_cnt[g] = inc(ta, "A", sA)

            def emit_ht(g):
                bg = BGS[g]
                ht = nc.vector.scalar_tensor_tensor(
                    sts[g][:, :], t1s[g].ap()[:, 4 * bg:5 * bg], 1.0,
                    tcs[g].ap()[:, :], ADD, MULT)
                ht._wait_ge(sA, t_cnt[g])
                ht_cnt[g] = inc(ht, "D", sD)

            for g in range(G):
                emit_cell(g)
            for g in range(G):
                emit_ht(g)

            if t == 0:
                # wipe layer-2 pollution from the bogus step -1
                for g in range(G):
                    bg = BGS[g]
                    m = st_all.ap()[64:128, BOFF[g]:BOFF[g] + BGS[g]]
                    inc(nc.vector.memset(m, 0.0), "D", sD)
                    inc(nc.vector.memset(
                        t1s[g].ap()[64:128, 2 * bg:3 * bg], 0.0), "D", sD)
                    ht_cnt[g] = cnt["D"]

        # output: 2*h2 = st rows 64:128, f16 (host halves and casts)
        od = nc.sync.dma_start(out_d.ap()[:, :], st_all.ap()[64:128, :])
        od._wait_ge(sD, cnt["D"])
        od.then_inc(sM, 16)
        cnt["M"] += 16
        nc.sync.wait_ge(sM, cnt["M"])
        nc.all_engine_barrier()

    nc.compile()
    return nc


def _get_program(t_steps=T):
    key = ("prog", t_steps)
    if key not in _CACHE:
        _CACHE[key] = build_program(t_steps)
    return _CACHE[key]


def make_in_map(inputs, core=0):
    x = np.asarray(inputs["x"], np.float32)
    t_steps = x.shape[1]
    wst, wx = _prep_weights(
        *(np.asarray(inputs[k], np.float32) for k in
          ("W_ih0", "W_hh0", "b_ih0", "b_hh0", "W_ih1", "W_hh1", "b_ih1",
           "b_hh1"))
    )
    xc = x[core * BL:(core + 1) * BL]
    xt = np.ones((4, 512 + t_steps * BL), np.float16)
    xt[:, 0:512] = wx
    xt[0:3, 512:] = xc.transpose(2, 1, 0).reshape(3, t_steps * BL).astype(
        np.float16)
    return {"xt": xt, "wst": wst}


def extract_out(out_mem):
    return out_mem.view(np.float16).reshape(64, BL).T.astype(np.float32) * 0.5


def kernel(x, W_ih0, W_hh0, b_ih0, b_hh0, W_ih1, W_hh1, b_ih1, b_hh1):
    from concourse import bass_utils

    inputs = dict(x=x, W_ih0=W_ih0, W_hh0=W_hh0, b_ih0=b_ih0, b_hh0=b_hh0,
                  W_ih1=W_ih1, W_hh1=W_hh1, b_ih1=b_ih1, b_hh1=b_hh1)
    nc = _get_program(T)
    in_maps = [make_in_map(inputs, core=c) for c in range(NCORES)]
    res = bass_utils.run_bass_kernel_spmd(nc, in_maps, core_ids=list(range(NCORES)))
    outs = [np.asarray(res.results[c]["out"]).T.astype(np.float32) * 0.5
            for c in range(NCORES)]
    return np.concatenate(outs, axis=0).astype(np.float32)


if __name__ == "__main__":
    rng = np.random.default_rng(0)
    s = 1.0 / np.sqrt(H)
    inputs = {
        "x": rng.standard_normal((B, T, I), np.float32),
        "W_ih0": rng.uniform(-s, s, (4 * H, I)).astype(np.float32),
        "W_hh0": rng.uniform(-s, s, (4 * H, H)).astype(np.float32),
        "b_ih0": rng.uniform(-s, s, 4 * H).astype(np.float32),
        "b_hh0": rng.uniform(-s, s, 4 * H).astype(np.float32),
        "W_ih1": rng.uniform(-s, s, (4 * H, H)).astype(np.float32),
        "W_hh1": rng.uniform(-s, s, (4 * H, H)).astype(np.float32),
        "b_ih1": rng.uniform(-s, s, 4 * H).astype(np.float32),
        "b_hh1": rng.uniform(-s, s, 4 * H).astype(np.float32),
    }
    out = kernel(**inputs)
    print(out.shape, out.dtype, np.abs(out).max())
